# revision 18
# baseline (speedup 1.0000x reference)
# Trainium2 Bass kernel for nn_CauRecNet (2-layer residual-cell LSTM scan).
#
# v2: batch-major scan. Pure data-parallel over 8 NeuronCores (16384 rows
# each). Host pre-transposes x to feature-major bf16 (xt) and cell_state to
# csT (f32, init path needs f32 per numerics ablation), so the device does
# zero layout work.
#
# Per core: 16 "pair tiles" of 1024 rows (two 512-halves stacked on the
# partition axis for L0, per-half for L1, as in v1: block-diagonal /
# half-masked weights). The t-loop is the OUTER For_i; inside one iteration
# all 16 pairs' cell updates run with a 2-pair software skew, so every
# engine queue holds ~16 independent dependency chains and ACT (the
# bottleneck: 5 transcendental evals per hidden element) stays saturated.
#
# States live in SBUF across the whole scan (bf16; updated in place).
# Gate PSUM f32; activations/intermediates bf16.

import numpy as np
import ml_dtypes

B, T, F = 131072, 15, 12
H1, H2, CS = 64, 128, 96
NCORES = 8
BL = B // NCORES          # 16384 rows per core
NT = 512                  # matmul free dim (one half)
NPAIR = BL // (2 * NT)    # 16 pair-tiles per core

BF16 = ml_dtypes.bfloat16

_BUILD_CACHE = {}


def _build_bass(has_gate_bias, has_vec_bias, repeat=1, sim_steps=None):
    # sim_steps: dev-only — replace the For_i(1,T) hardware loop with a
    # python-unrolled loop of that many steps (TimelineSim can't handle
    # register branches). Not used by kernel().
    import contextlib
    import concourse.bacc as bacc
    import concourse.tile as tile
    from concourse import mybir

    f32 = mybir.dt.float32
    bf16 = mybir.dt.bfloat16
    AF = mybir.ActivationFunctionType

    nc = bacc.Bacc()

    # ---- DRAM I/O ----
    xt_d = nc.dram_tensor("xt", [2 * F, T, NPAIR * NT], bf16, kind="ExternalInput")
    csT_d = nc.dram_tensor("csT", [NPAIR, CS, 2 * NT], f32, kind="ExternalInput")
    w0ih_d = nc.dram_tensor("w0ih_bd", [2 * F, 512], bf16, kind="ExternalInput")
    w0hh_d = nc.dram_tensor("w0hh_bd", [2 * H1, 512], bf16, kind="ExternalInput")
    w1ihA_d = nc.dram_tensor("w1ih_A", [2 * H1, 512], bf16, kind="ExternalInput")
    w1ihB_d = nc.dram_tensor("w1ih_B", [2 * H1, 512], bf16, kind="ExternalInput")
    w1hh_d = nc.dram_tensor("w1hhT", [H2, 512], bf16, kind="ExternalInput")
    fc1A_d = nc.dram_tensor("fc1_A", [CS, 2 * H1], f32, kind="ExternalInput")
    fc1B_d = nc.dram_tensor("fc1_B", [CS, 2 * H1], f32, kind="ExternalInput")
    fc2_d = nc.dram_tensor("fc2T", [CS, H2], f32, kind="ExternalInput")
    d1_d = nc.dram_tensor("d1T", [H2, H1], bf16, kind="ExternalInput")
    d2_d = nc.dram_tensor("d2T", [H1, 1], bf16, kind="ExternalInput")
    gb_d = nc.dram_tensor("gate_bias", [128, 8], f32, kind="ExternalInput")
    vb_d = nc.dram_tensor("vec_bias", [128, 4], f32, kind="ExternalInput")
    pred_d = nc.dram_tensor("pred", [BL, 1], f32, kind="ExternalOutput")

    # xt_view[t] = [2F, (pair, 512)]
    xt_view = xt_d[:].rearrange("f t q -> t f q")
    # steps 1..14 as 7 x 2 for the double-step loop: xt_pairs[k][u] = t=1+2k+u
    xt_pairs = xt_d[:, 1:T, :].rearrange("f (s u) q -> s u f q", u=2)
    pred_view = pred_d[:].rearrange("(p h n) o -> p h o n", h=2, n=NT)

    with tile.TileContext(nc) as tc:
        ctx = contextlib.ExitStack()
        with ctx:
            consts = ctx.enter_context(tc.tile_pool(name="consts", bufs=1))
            csts = ctx.enter_context(tc.tile_pool(name="csts", bufs=1))
            xts = ctx.enter_context(tc.tile_pool(name="xts", bufs=2))
            states = ctx.enter_context(tc.tile_pool(name="states", bufs=1))
            scr = ctx.enter_context(tc.tile_pool(name="scr", bufs=2))
            outp = ctx.enter_context(tc.tile_pool(name="outp", bufs=4))
            pp = ctx.enter_context(tc.tile_pool(name="pp", bufs=2, space="PSUM"))

            def load_const(name, dram, shape, dt):
                t = consts.tile(shape, dt, name=name)
                nc.sync.dma_start(out=t, in_=dram[:])
                return t

            w0ih = load_const("w0ih", w0ih_d, [2 * F, 512], bf16)
            w0hh = load_const("w0hh", w0hh_d, [2 * H1, 512], bf16)
            w1ihA = load_const("w1ihA", w1ihA_d, [2 * H1, 512], bf16)
            w1ihB = load_const("w1ihB", w1ihB_d, [2 * H1, 512], bf16)
            w1hh = load_const("w1hh", w1hh_d, [H2, 512], bf16)
            fc1A = load_const("fc1A", fc1A_d, [CS, 128], f32)
            fc1B = load_const("fc1B", fc1B_d, [CS, 128], f32)
            fc2 = load_const("fc2", fc2_d, [CS, H2], f32)
            d1w = load_const("d1w", d1_d, [H2, H1], bf16)
            d2w = load_const("d2w", d2_d, [H1, 1], bf16)
            gbias = load_const("gbias", gb_d, [128, 8], f32)
            vbias = load_const("vbias", vb_d, [128, 4], f32)

            # ---- persistent per-pair states (bf16, in-place updates) ----
            st = {}
            for p in range(NPAIR):
                for k, rows in (("c0", 128), ("h0", 128),
                                ("c1A", H2), ("c1B", H2),
                                ("h1A", H2), ("h1B", H2)):
                    st[f"{k}_{p}"] = states.tile(
                        [rows, NT], bf16, tag=f"{k}_{p}", name=f"{k}_{p}")

            def body():
                # ---------- init: c0 = fc1@cs, c1 = fc2@cs (f32 path) ----------
                for p in range(NPAIR):
                    cst = csts.tile([CS, 2 * NT], f32, tag="cst", name="cst")
                    nc.sync.dma_start(out=cst, in_=csT_d[p])
                    ip = pp.tile([128, 2048], f32, tag="G", name="ip")
                    nc.tensor.matmul(ip[:, 0:NT], fc1A, cst[:, 0:NT],
                                     start=True, stop=False)
                    nc.tensor.matmul(ip[:, 0:NT], fc1B, cst[:, NT:2 * NT],
                                     start=False, stop=True)
                    nc.tensor.matmul(ip[:, NT:2 * NT], fc2, cst[:, 0:NT],
                                     start=True, stop=True)
                    nc.tensor.matmul(ip[:, 2 * NT:3 * NT], fc2, cst[:, NT:2 * NT],
                                     start=True, stop=True)
                    if has_vec_bias:
                        nc.vector.tensor_scalar_add(st[f"c0_{p}"], ip[:, 0:NT],
                                                    vbias[:, 0:1])
                        nc.vector.tensor_scalar_add(st[f"c1A_{p}"], ip[:, NT:2 * NT],
                                                    vbias[:, 1:2])
                        nc.vector.tensor_scalar_add(st[f"c1B_{p}"], ip[:, 2 * NT:3 * NT],
                                                    vbias[:, 1:2])
                    else:
                        # split across DVE and the otherwise-idle ACT engine
                        nc.vector.tensor_copy(out=st[f"c0_{p}"], in_=ip[:, 0:NT])
                        nc.scalar.activation(st[f"c1A_{p}"], ip[:, NT:2 * NT],
                                             AF.Copy)
                        nc.scalar.activation(st[f"c1B_{p}"], ip[:, 2 * NT:3 * NT],
                                             AF.Copy)

                # ---------- one scan step over all pairs ----------
                # Phases are split front/tail and software-skewed so the ACT
                # queue never head-of-line blocks on a DVE-produced cres:
                #   slot s: l0_front(s), l0_tail(s-1), l1_front(s-3), l1_tail(s-4)
                def l0_front(p, xt, first):
                    G = pp.tile([128, 2048], f32, tag="G", name="G0")
                    x_p = xt[:, p * NT:(p + 1) * NT]
                    for g in range(4):
                        reg = G[:, g * NT:(g + 1) * NT]
                        nc.tensor.matmul(reg, w0ih[:, g * 128:(g + 1) * 128],
                                         x_p, start=True, stop=first)
                        if not first:
                            nc.tensor.matmul(reg, w0hh[:, g * 128:(g + 1) * 128],
                                             st[f"h0_{p}"], start=False, stop=True)
                    if has_gate_bias:
                        for g in range(4):
                            nc.vector.tensor_scalar_add(
                                G[:, g * NT:(g + 1) * NT],
                                G[:, g * NT:(g + 1) * NT], gbias[:, g:g + 1])
                    sg = scr.tile([128, 1536], bf16, tag="sg0", name="sg0")
                    nc.scalar.activation(sg, G[:, 0:1536], AF.Sigmoid)
                    gt = scr.tile([128, NT], bf16, tag="gt0", name="gt0")
                    nc.scalar.activation(gt, G[:, 1536:2048], AF.Tanh)
                    c0 = st[f"c0_{p}"]
                    t1 = scr.tile([128, NT], bf16, tag="t1_0", name="t1_0")
                    nc.vector.tensor_mul(t1, sg[:, NT:2 * NT], c0)
                    t2 = scr.tile([128, NT], bf16, tag="t2_0", name="t2_0")
                    nc.vector.tensor_mul(t2, sg[:, 0:NT], gt)
                    cres = scr.tile([128, NT], bf16, tag="cres0", name="cres0")
                    nc.vector.tensor_add(cres, t1, t2)
                    nc.vector.tensor_add(c0, c0, cres)
                    return sg, cres

                def l0_tail(p, sg, cres):
                    tc_ = scr.tile([128, NT], bf16, tag="tc0", name="tc0")
                    nc.scalar.activation(tc_, cres, AF.Tanh)
                    nc.vector.tensor_mul(st[f"h0_{p}"], sg[:, 2 * NT:3 * NT], tc_)

                def l1_front(p, first):
                    cres1 = scr.tile([128, 2 * NT], bf16, tag="cres1", name="cres1")
                    sgs = []
                    for hf, wih in ((0, w1ihA), (1, w1ihB)):
                        hn = "AB"[hf]
                        G = pp.tile([128, 2048], f32, tag="G", name="G1")
                        for g in range(4):
                            reg = G[:, g * NT:(g + 1) * NT]
                            nc.tensor.matmul(reg, wih[:, g * 128:(g + 1) * 128],
                                             st[f"h0_{p}"], start=True, stop=first)
                            if not first:
                                nc.tensor.matmul(reg, w1hh[:, g * 128:(g + 1) * 128],
                                                 st[f"h1{hn}_{p}"],
                                                 start=False, stop=True)
                        if has_gate_bias:
                            for g in range(4):
                                nc.vector.tensor_scalar_add(
                                    G[:, g * NT:(g + 1) * NT],
                                    G[:, g * NT:(g + 1) * NT], gbias[:, 4 + g:5 + g])
                        sg = scr.tile([128, 1536], bf16, tag=f"sg1{hn}", name="sg1")
                        nc.scalar.activation(sg, G[:, 0:1536], AF.Sigmoid)
                        gt = scr.tile([128, NT], bf16, tag=f"gt1{hn}", name="gt1")
                        nc.scalar.activation(gt, G[:, 1536:2048], AF.Tanh)
                        c1 = st[f"c1{hn}_{p}"]
                        t1 = scr.tile([128, NT], bf16, tag=f"t11{hn}", name="t11")
                        nc.vector.tensor_mul(t1, sg[:, NT:2 * NT], c1)
                        t2 = scr.tile([128, NT], bf16, tag=f"t21{hn}", name="t21")
                        nc.vector.tensor_mul(t2, sg[:, 0:NT], gt)
                        cr = cres1[:, hf * NT:(hf + 1) * NT]
                        nc.vector.tensor_add(cr, t1, t2)
                        nc.vector.tensor_add(c1, c1, cr)
                        sgs.append(sg)
                    return sgs, cres1

                def l1_tail(p, sgs, cres1):
                    tc1 = scr.tile([128, 2 * NT], bf16, tag="tc1", name="tc1")
                    nc.scalar.activation(tc1, cres1, AF.Tanh)
                    nc.vector.tensor_mul(st[f"h1A_{p}"], sgs[0][:, 2 * NT:3 * NT],
                                         tc1[:, 0:NT])
                    nc.vector.tensor_mul(st[f"h1B_{p}"], sgs[1][:, 2 * NT:3 * NT],
                                         tc1[:, NT:2 * NT])

                def step(xt, first):
                    fr0, fr1 = {}, {}
                    for s in range(NPAIR + 4):
                        if s < NPAIR:
                            fr0[s] = l0_front(s, xt, first)
                        if 1 <= s <= NPAIR:
                            l0_tail(s - 1, *fr0.pop(s - 1))
                        if 3 <= s < NPAIR + 3:
                            fr1[s - 3] = l1_front(s - 3, first)
                        if s >= 4:
                            l1_tail(s - 4, *fr1.pop(s - 4))

                # t = 0 unrolled (h == 0: ih-only matmuls, no memset needed)
                xt0 = xts.tile([2 * F, NPAIR * NT], bf16, tag="xt", name="xt0")
                nc.sync.dma_start(out=xt0, in_=xt_view[0])
                step(xt0, True)

                def double_step(src_a, src_b):
                    xt_a = xts.tile([2 * F, NPAIR * NT], bf16, tag="xt",
                                    name="xt_a")
                    nc.sync.dma_start(out=xt_a, in_=src_a)
                    xt_b = xts.tile([2 * F, NPAIR * NT], bf16, tag="xt",
                                    name="xt_b")
                    nc.sync.dma_start(out=xt_b, in_=src_b)
                    step(xt_a, False)
                    step(xt_b, False)

                if sim_steps is not None:
                    for k in range(sim_steps // 2):
                        double_step(xt_pairs[k][0], xt_pairs[k][1])
                else:
                    with tc.For_i(0, (T - 1) // 2, 1,
                                  hint_engines=(nc.tensor.engine, nc.vector.engine,
                                                nc.scalar.engine)) as k:
                        double_step(xt_pairs[k][0], xt_pairs[k][1])

                # ---------- head ----------
                for p in range(NPAIR):
                    hp = pp.tile([128, 2048], f32, tag="G", name="hp")
                    for hf in range(2):
                        hn = "AB"[hf]
                        nc.tensor.matmul(hp[0:H1, hf * NT:(hf + 1) * NT], d1w,
                                         st[f"h1{hn}_{p}"], start=True, stop=True)
                        z = outp.tile([H1, NT], bf16, tag="z", name="z")
                        if has_vec_bias:
                            nc.vector.tensor_scalar_add(
                                z, hp[0:H1, hf * NT:(hf + 1) * NT], vbias[0:H1, 2:3])
                        elif hf == 0:
                            nc.scalar.activation(
                                z, hp[0:H1, hf * NT:(hf + 1) * NT], AF.Copy)
                        else:
                            nc.vector.tensor_copy(
                                out=z, in_=hp[0:H1, hf * NT:(hf + 1) * NT])
                        nc.tensor.matmul(hp[0:1, 1024 + hf * NT:1024 + (hf + 1) * NT],
                                         d2w, z, start=True, stop=True)
                        out_sb = outp.tile([1, NT], f32, tag="out_sb",
                                           name="out_sb")
                        if has_vec_bias:
                            nc.vector.tensor_scalar_add(
                                out_sb, hp[0:1, 1024 + hf * NT:1024 + (hf + 1) * NT],
                                vbias[0:1, 3:4])
                        elif hf == 1:
                            nc.scalar.activation(
                                out_sb, hp[0:1, 1024 + hf * NT:1024 + (hf + 1) * NT],
                                AF.Copy)
                        else:
                            nc.vector.tensor_copy(
                                out=out_sb,
                                in_=hp[0:1, 1024 + hf * NT:1024 + (hf + 1) * NT])
                        nc.sync.dma_start(out=pred_view[p][hf], in_=out_sb)

            if repeat == 1:
                body()
            else:
                with tc.For_i(0, repeat, 1):
                    body()

    nc.finalize()
    return nc


def _get_nc(key):
    if key not in _BUILD_CACHE:
        _BUILD_CACHE[key] = _build_bass(*key)
    return _BUILD_CACHE[key]


def _prep_weights(inputs):
    # gate order permutation i,f,g,o -> i,f,o,g (sigmoid gates contiguous)
    def perm(n):
        return np.concatenate([np.arange(0, 2 * n), np.arange(3 * n, 4 * n),
                               np.arange(2 * n, 3 * n)])
    p0, p1 = perm(H1), perm(H2)

    w0ihT = inputs["l0_w_ih"][p0].T.astype(np.float32)     # [12, 256]
    w0hhT = inputs["l0_w_hh"][p0].T.astype(np.float32)     # [64, 256]
    w1ihT = inputs["l1_w_ih"][p1].T.astype(np.float32)     # [64, 512]
    w1hhT = inputs["l1_w_hh"][p1].T.astype(np.float32)     # [128, 512]

    # L0 ih block-diagonal, rows interleaved (f,half)
    w0ih_bd = np.zeros((2 * F, 512), np.float32)
    w0ih_bd[0::2, :] = np.concatenate(
        [np.pad(w0ihT[:, g * 64:(g + 1) * 64], [(0, 0), (0, 64)])
         for g in range(4)], axis=1)
    w0ih_bd[1::2, :] = np.concatenate(
        [np.pad(w0ihT[:, g * 64:(g + 1) * 64], [(0, 0), (64, 0)])
         for g in range(4)], axis=1)
    # L0 hh block-diagonal (A rows 0:64, B rows 64:128)
    w0hh_bd = np.zeros((2 * H1, 512), np.float32)
    for g in range(4):
        blk = w0hhT[:, g * 64:(g + 1) * 64]
        w0hh_bd[0:64, g * 128:g * 128 + 64] = blk
        w0hh_bd[64:128, g * 128 + 64:(g + 1) * 128] = blk
    # L1 ih half-masked (reads stacked h0)
    w1ih_A = np.concatenate([w1ihT, np.zeros_like(w1ihT)], axis=0)   # [128, 512]
    w1ih_B = np.concatenate([np.zeros_like(w1ihT), w1ihT], axis=0)
    fc1T = inputs["fc1_w"].T.astype(np.float32)            # [96, 64]
    fc1_A = np.concatenate([fc1T, np.zeros_like(fc1T)], axis=1)      # [96, 128]
    fc1_B = np.concatenate([np.zeros_like(fc1T), fc1T], axis=1)

    wm = {
        "w0ih_bd": w0ih_bd.astype(BF16),
        "w0hh_bd": w0hh_bd.astype(BF16),
        "w1ih_A": w1ih_A.astype(BF16),
        "w1ih_B": w1ih_B.astype(BF16),
        "w1hhT": np.ascontiguousarray(w1hhT).astype(BF16),
        "fc1_A": fc1_A,
        "fc1_B": fc1_B,
        "fc2T": np.ascontiguousarray(inputs["fc2_w"].T).astype(np.float32),
        "d1T": np.ascontiguousarray(inputs["d1_w"].T).astype(BF16),
        "d2T": np.ascontiguousarray(inputs["d2_w"].T).astype(BF16),
    }

    b0 = (inputs["l0_b_ih"] + inputs["l0_b_hh"]).astype(np.float32)[p0]   # [256]
    b1 = (inputs["l1_b_ih"] + inputs["l1_b_hh"]).astype(np.float32)[p1]   # [512]
    gb = np.zeros((128, 8), np.float32)
    for g in range(4):
        gb[:, g] = np.tile(b0[g * 64:(g + 1) * 64], 2)     # stacked [A;B]
        gb[:, 4 + g] = b1[g * 128:(g + 1) * 128]
    vb = np.zeros((128, 4), np.float32)
    vb[:, 0] = np.tile(inputs["fc1_b"], 2)
    vb[:, 1] = inputs["fc2_b"]
    vb[0:H1, 2] = inputs["d1_b"]
    vb[0:1, 3] = inputs["d2_b"]
    wm["gate_bias"] = gb
    wm["vec_bias"] = vb
    has_gate_bias = bool(np.any(b0) or np.any(b1))
    has_vec_bias = bool(np.any(vb))
    return wm, has_gate_bias, has_vec_bias


def _in_maps(inputs, wm):
    x = np.asarray(inputs["input_seq"], dtype=np.float32)
    cs = np.asarray(inputs["cell_state"], dtype=np.float32)
    # xt[2f+h, t, p*512+n] = x[p*1024 + h*512 + n, t, f]  (per core)
    xc = np.ascontiguousarray(x).reshape(NCORES, NPAIR, 2, NT, T, F)
    xt = np.ascontiguousarray(
        xc.transpose(0, 5, 2, 4, 1, 3)                      # [c,f,h,t,p,n]
    ).reshape(NCORES, 2 * F, T, NPAIR * NT).astype(BF16)
    # csT[p, k, h*512+n] = cs[p*1024 + h*512 + n, k]
    cc = cs.reshape(NCORES, NPAIR, 2, NT, CS)
    csT = np.ascontiguousarray(cc.transpose(0, 1, 4, 2, 3)).reshape(
        NCORES, NPAIR, CS, 2 * NT)
    maps = []
    for i in range(NCORES):
        m = dict(wm)
        m["xt"] = np.ascontiguousarray(xt[i])
        m["csT"] = np.ascontiguousarray(csT[i])
        maps.append(m)
    return maps


def kernel(**inputs):
    inputs = {k: np.asarray(v) for k, v in inputs.items()}
    wm, hgb, hvb = _prep_weights(inputs)
    nc = _get_nc((hgb, hvb))
    from concourse.bass_utils import run_bass_kernel_spmd
    res = run_bass_kernel_spmd(nc, _in_maps(inputs, wm),
                               core_ids=list(range(NCORES)))
    return np.concatenate([r["pred"] for r in res.results], axis=0)


# revision 44
# speedup vs baseline: 1.3287x; 1.3287x over previous
# Trainium2 Bass kernel for nn_CauRecNet (2-layer residual-cell LSTM scan).
#
# v2: batch-major scan. Pure data-parallel over 8 NeuronCores (16384 rows
# each). Host pre-transposes x to feature-major bf16 (xt) and cell_state to
# csT (f32, init path needs f32 per numerics ablation), so the device does
# zero layout work.
#
# Per core: 16 "pair tiles" of 1024 rows (two 512-halves stacked on the
# partition axis for L0, per-half for L1, as in v1: block-diagonal /
# half-masked weights). The t-loop is the OUTER For_i; inside one iteration
# all 16 pairs' cell updates run with a 2-pair software skew, so every
# engine queue holds ~16 independent dependency chains and ACT (the
# bottleneck: 5 transcendental evals per hidden element) stays saturated.
#
# States live in SBUF across the whole scan (bf16; updated in place).
# Gate PSUM f32; activations/intermediates bf16.

import numpy as np
import ml_dtypes

B, T, F = 131072, 15, 12
H1, H2, CS = 64, 128, 96
NCORES = 8
BL = B // NCORES          # 16384 rows per core
NT = 512                  # matmul free dim (one half)
NPAIR = BL // (2 * NT)    # 16 pair-tiles per core

BF16 = ml_dtypes.bfloat16

_BUILD_CACHE = {}


def _build_bass(has_gate_bias, has_vec_bias, repeat=1, sim_steps=None,
                double_step=True, tail_split=True, act_copy=False,
                unroll=True):
    # sim_steps: dev-only — replace the For_i(1,T) hardware loop with a
    # python-unrolled loop of that many steps (TimelineSim can't handle
    # register branches). Not used by kernel().
    # double_step/tail_split/act_copy: dev-only A/B toggles.
    double_step_on = double_step
    import contextlib
    import concourse.bacc as bacc
    import concourse.tile as tile
    from concourse import mybir

    f32 = mybir.dt.float32
    bf16 = mybir.dt.bfloat16
    AF = mybir.ActivationFunctionType

    nc = bacc.Bacc()

    # ---- DRAM I/O ----
    xt_d = nc.dram_tensor("xt", [2 * 2 * F, T, 8 * NT], bf16, kind="ExternalInput")
    csT_d = nc.dram_tensor("csT", [NPAIR, CS, 2 * NT], f32, kind="ExternalInput")
    w0ih_d = nc.dram_tensor("w0ih_bd", [128, 512], bf16, kind="ExternalInput")
    w0hh_d = nc.dram_tensor("w0hh_bd", [2 * H1, 512], bf16, kind="ExternalInput")
    w1ihA_d = nc.dram_tensor("w1ih_A", [2 * H1, 512], bf16, kind="ExternalInput")
    w1ihB_d = nc.dram_tensor("w1ih_B", [2 * H1, 512], bf16, kind="ExternalInput")
    w1hh_d = nc.dram_tensor("w1hhT", [H2, 512], bf16, kind="ExternalInput")
    fc1A_d = nc.dram_tensor("fc1_A", [CS, 2 * H1], f32, kind="ExternalInput")
    fc1B_d = nc.dram_tensor("fc1_B", [CS, 2 * H1], f32, kind="ExternalInput")
    fc2_d = nc.dram_tensor("fc2T", [CS, H2], f32, kind="ExternalInput")
    d1_d = nc.dram_tensor("d1T", [H2, H1], bf16, kind="ExternalInput")
    d2_d = nc.dram_tensor("d2T", [H1, 1], bf16, kind="ExternalInput")
    gb_d = nc.dram_tensor("gate_bias", [128, 8], f32, kind="ExternalInput")
    vb_d = nc.dram_tensor("vec_bias", [128, 4], f32, kind="ExternalInput")
    pred_d = nc.dram_tensor("pred", [BL, 1], f32, kind="ExternalOutput")

    # xt_view[t] = [(group, 2F), (pairblock, 512)]
    xt_view = xt_d[:].rearrange("(g f) t q -> t g f q", g=2)
    # steps 1..14 as 7 x 2 for the double-step loop: xt_pairs[k][u] = t=1+2k+u
    xt_pairs = xt_d[:, 1:T, :].rearrange("(g f) (s u) q -> s u g f q", g=2, u=2)
    pred_view = pred_d[:].rearrange("(p h n) o -> p h o n", h=2, n=NT)

    with tile.TileContext(nc) as tc:
        ctx = contextlib.ExitStack()
        with ctx:
            consts = ctx.enter_context(tc.tile_pool(name="consts", bufs=1))
            csts = ctx.enter_context(tc.tile_pool(name="csts", bufs=3))
            xts = ctx.enter_context(tc.tile_pool(name="xts", bufs=2))
            states = ctx.enter_context(tc.tile_pool(name="states", bufs=1))
            scr = ctx.enter_context(tc.tile_pool(name="scr", bufs=2))
            outp = ctx.enter_context(tc.tile_pool(name="outp", bufs=2))
            pp = ctx.enter_context(tc.tile_pool(name="pp", bufs=2, space="PSUM"))

            def load_const(name, dram, shape, dt):
                t = consts.tile(shape, dt, name=name)
                nc.sync.dma_start(out=t, in_=dram[:])
                return t

            w0ih = load_const("w0ih", w0ih_d, [128, 512], bf16)
            w0hh = load_const("w0hh", w0hh_d, [2 * H1, 512], bf16)
            w1ihA = load_const("w1ihA", w1ihA_d, [2 * H1, 512], bf16)
            w1ihB = load_const("w1ihB", w1ihB_d, [2 * H1, 512], bf16)
            w1hh = load_const("w1hh", w1hh_d, [H2, 512], bf16)
            fc1A = load_const("fc1A", fc1A_d, [CS, 128], f32)
            fc1B = load_const("fc1B", fc1B_d, [CS, 128], f32)
            fc2 = load_const("fc2", fc2_d, [CS, H2], f32)
            d1w = load_const("d1w", d1_d, [H2, H1], bf16)
            d2w = load_const("d2w", d2_d, [H1, 1], bf16)
            gbias = load_const("gbias", gb_d, [128, 8], f32)
            vbias = load_const("vbias", vb_d, [128, 4], f32)

            # ---- persistent per-pair states (bf16, in-place updates) ----
            st = {}
            for p in range(NPAIR):
                for k, rows in (("c0", 128), ("h0", 128),
                                ("c1A", H2), ("c1B", H2),
                                ("h1A", H2), ("h1B", H2)):
                    st[f"{k}_{p}"] = states.tile(
                        [rows, NT], bf16, tag=f"{k}_{p}", name=f"{k}_{p}")

            def body():
                # ---------- init pieces (interleaved into the t=0 step) ----------
                cst_tiles = {}

                def emit_cst_dma(p):
                    cst = csts.tile([CS, 2 * NT], f32, tag="cst", name="cst")
                    nc.sync.dma_start(out=cst, in_=csT_d[p])
                    cst_tiles[p] = cst

                def emit_init(p):
                    cst = cst_tiles.pop(p)
                    ip = pp.tile([128, 2048], f32, tag="G", name="ip")
                    nc.tensor.matmul(ip[:, 0:NT], fc1A, cst[:, 0:NT],
                                     start=True, stop=False)
                    nc.tensor.matmul(ip[:, 0:NT], fc1B, cst[:, NT:2 * NT],
                                     start=False, stop=True)
                    nc.tensor.matmul(ip[:, NT:2 * NT], fc2, cst[:, 0:NT],
                                     start=True, stop=True)
                    nc.tensor.matmul(ip[:, 2 * NT:3 * NT], fc2, cst[:, NT:2 * NT],
                                     start=True, stop=True)
                    if has_vec_bias:
                        nc.vector.tensor_scalar_add(st[f"c0_{p}"], ip[:, 0:NT],
                                                    vbias[:, 0:1])
                        nc.vector.tensor_scalar_add(st[f"c1A_{p}"], ip[:, NT:2 * NT],
                                                    vbias[:, 1:2])
                        nc.vector.tensor_scalar_add(st[f"c1B_{p}"], ip[:, 2 * NT:3 * NT],
                                                    vbias[:, 1:2])
                    elif act_copy:
                        # split across DVE and the otherwise-idle ACT engine
                        nc.vector.tensor_copy(out=st[f"c0_{p}"], in_=ip[:, 0:NT])
                        nc.scalar.activation(st[f"c1A_{p}"], ip[:, NT:2 * NT],
                                             AF.Copy)
                        nc.scalar.activation(st[f"c1B_{p}"], ip[:, 2 * NT:3 * NT],
                                             AF.Copy)
                    else:
                        nc.vector.tensor_copy(out=st[f"c0_{p}"], in_=ip[:, 0:NT])
                        nc.vector.tensor_copy(out=st[f"c1A_{p}"], in_=ip[:, NT:2 * NT])
                        nc.vector.tensor_copy(out=st[f"c1B_{p}"], in_=ip[:, 2 * NT:3 * NT])

                def head(p):
                    hp = pp.tile([128, 2048], f32, tag="G", name="hp")
                    for hf in range(2):
                        hn = "AB"[hf]
                        nc.tensor.matmul(hp[0:H1, hf * NT:(hf + 1) * NT], d1w,
                                         st[f"h1{hn}_{p}"], start=True, stop=True)
                        z = outp.tile([H1, NT], bf16, tag="z", name="z")
                        if has_vec_bias:
                            nc.vector.tensor_scalar_add(
                                z, hp[0:H1, hf * NT:(hf + 1) * NT], vbias[0:H1, 2:3])
                        elif act_copy and hf == 0:
                            nc.scalar.activation(
                                z, hp[0:H1, hf * NT:(hf + 1) * NT], AF.Copy)
                        else:
                            nc.vector.tensor_copy(
                                out=z, in_=hp[0:H1, hf * NT:(hf + 1) * NT])
                        nc.tensor.matmul(hp[0:1, 1024 + hf * NT:1024 + (hf + 1) * NT],
                                         d2w, z, start=True, stop=True)
                        out_sb = outp.tile([1, NT], f32, tag="out_sb",
                                           name="out_sb")
                        if has_vec_bias:
                            nc.vector.tensor_scalar_add(
                                out_sb, hp[0:1, 1024 + hf * NT:1024 + (hf + 1) * NT],
                                vbias[0:1, 3:4])
                        elif act_copy and hf == 1:
                            nc.scalar.activation(
                                out_sb, hp[0:1, 1024 + hf * NT:1024 + (hf + 1) * NT],
                                AF.Copy)
                        else:
                            nc.vector.tensor_copy(
                                out=out_sb,
                                in_=hp[0:1, 1024 + hf * NT:1024 + (hf + 1) * NT])
                        nc.sync.dma_start(out=pred_view[p][hf], in_=out_sb)

                # ---------- one scan step over all pairs ----------
                # Phases are split front/tail and software-skewed so the ACT
                # queue never head-of-line blocks on a DVE-produced cres.
                # cres/tanh(cres) are merged across pair-PAIRS (q = 2 pairs)
                # for fewer, larger ACT instructions.
                def l0_front(p, xt, first, cr, last):
                    G = pp.tile([128, 2048], f32, tag="G", name="G0")
                    gb_, cb = 64 * (p % 2), (p // 2) * NT
                    x_p = xt[gb_:gb_ + 2 * F, cb:cb + NT]
                    for g in range(4):
                        reg = G[:, g * NT:(g + 1) * NT]
                        nc.tensor.matmul(reg,
                                         w0ih[gb_:gb_ + 2 * F,
                                              g * 128:(g + 1) * 128],
                                         x_p, start=True, stop=first)
                        if not first:
                            nc.tensor.matmul(reg, w0hh[:, g * 128:(g + 1) * 128],
                                             st[f"h0_{p}"], start=False, stop=True)
                    if has_gate_bias:
                        for g in range(4):
                            nc.vector.tensor_scalar_add(
                                G[:, g * NT:(g + 1) * NT],
                                G[:, g * NT:(g + 1) * NT], gbias[:, g:g + 1])
                    sg = scr.tile([128, 1536], bf16, tag="sg0", name="sg0", bufs=4)
                    nc.scalar.activation(sg, G[:, 0:1536], AF.Sigmoid)
                    gt = scr.tile([128, NT], bf16, tag="gt0", name="gt0")
                    nc.scalar.activation(gt, G[:, 1536:2048], AF.Tanh)
                    c0 = st[f"c0_{p}"]
                    t1 = scr.tile([128, NT], bf16, tag="t1_0", name="t1_0")
                    nc.vector.tensor_mul(t1, sg[:, NT:2 * NT], c0)
                    t2 = scr.tile([128, NT], bf16, tag="t2_0", name="t2_0")
                    nc.vector.tensor_mul(t2, sg[:, 0:NT], gt)
                    nc.vector.tensor_add(cr, t1, t2)
                    if not last:
                        nc.vector.tensor_add(c0, c0, cr)
                    return sg

                def l0_tail2(q, sgA, sgB, cr0q):
                    tcq = scr.tile([128, 2 * NT], bf16, tag="tc0", name="tc0")
                    nc.scalar.activation(tcq, cr0q, AF.Tanh)
                    nc.vector.tensor_mul(st[f"h0_{2 * q}"], sgA[:, 2 * NT:3 * NT],
                                         tcq[:, 0:NT])
                    nc.vector.tensor_mul(st[f"h0_{2 * q + 1}"], sgB[:, 2 * NT:3 * NT],
                                         tcq[:, NT:2 * NT])

                def l1_front(p, first, last):
                    cres1 = scr.tile([128, 2 * NT], bf16, tag="cres1", name="cres1", bufs=3)
                    sgs = []
                    for hf, wih in ((0, w1ihA), (1, w1ihB)):
                        hn = "AB"[hf]
                        G = pp.tile([128, 2048], f32, tag="G", name="G1")
                        for g in range(4):
                            reg = G[:, g * NT:(g + 1) * NT]
                            nc.tensor.matmul(reg, wih[:, g * 128:(g + 1) * 128],
                                             st[f"h0_{p}"], start=True, stop=first)
                            if not first:
                                nc.tensor.matmul(reg, w1hh[:, g * 128:(g + 1) * 128],
                                                 st[f"h1{hn}_{p}"],
                                                 start=False, stop=True)
                        if has_gate_bias:
                            for g in range(4):
                                nc.vector.tensor_scalar_add(
                                    G[:, g * NT:(g + 1) * NT],
                                    G[:, g * NT:(g + 1) * NT], gbias[:, 4 + g:5 + g])
                        sg = scr.tile([128, 1536], bf16, tag=f"sg1{hn}", name="sg1", bufs=3)
                        nc.scalar.activation(sg, G[:, 0:1536], AF.Sigmoid)
                        gt = scr.tile([128, NT], bf16, tag=f"gt1{hn}", name="gt1")
                        nc.scalar.activation(gt, G[:, 1536:2048], AF.Tanh)
                        c1 = st[f"c1{hn}_{p}"]
                        t1 = scr.tile([128, NT], bf16, tag=f"t11{hn}", name="t11")
                        nc.vector.tensor_mul(t1, sg[:, NT:2 * NT], c1)
                        t2 = scr.tile([128, NT], bf16, tag=f"t21{hn}", name="t21")
                        nc.vector.tensor_mul(t2, sg[:, 0:NT], gt)
                        cr = cres1[:, hf * NT:(hf + 1) * NT]
                        nc.vector.tensor_add(cr, t1, t2)
                        if not last:
                            nc.vector.tensor_add(c1, c1, cr)
                        sgs.append(sg)
                    return sgs, cres1

                def l1_tail(p, sgs, cres1):
                    tc1 = scr.tile([128, 2 * NT], bf16, tag="tc1", name="tc1")
                    nc.scalar.activation(tc1, cres1, AF.Tanh)
                    nc.vector.tensor_mul(st[f"h1A_{p}"], sgs[0][:, 2 * NT:3 * NT],
                                         tc1[:, 0:NT])
                    nc.vector.tensor_mul(st[f"h1B_{p}"], sgs[1][:, 2 * NT:3 * NT],
                                         tc1[:, NT:2 * NT])

                def step(xt, first, last=False, init=False):
                    fr0, fr1 = {}, {}
                    cr0 = {}
                    if init:
                        for pre in range(3):
                            emit_cst_dma(pre)
                    for s in range(NPAIR + 6):
                        if s < NPAIR:
                            if init:
                                if s + 3 < NPAIR:
                                    emit_cst_dma(s + 3)
                                emit_init(s)
                            q, par = divmod(s, 2)
                            if par == 0:
                                cr0[q] = scr.tile([128, 2 * NT], bf16,
                                                  tag="cres0", name="cres0", bufs=3)
                            fr0[s] = l0_front(s, xt, first,
                                              cr0[q][:, par * NT:(par + 1) * NT],
                                              last)
                        if s >= 3 and s % 2 == 1 and s - 3 < NPAIR:
                            q = (s - 3) // 2
                            l0_tail2(q, fr0.pop(2 * q), fr0.pop(2 * q + 1),
                                     cr0.pop(q))
                        if 3 <= s < NPAIR + 3:
                            p = s - 3
                            fr1[p] = l1_front(p, first, last)
                        if 5 <= s < NPAIR + 5:
                            p = s - 5
                            l1_tail(p, *fr1.pop(p))
                            if last:
                                head(p)

                # t = 0 unrolled (h == 0: ih-only matmuls, no memset needed);
                # c-state init phases interleave into its slots
                def xt_dma(tile_, src_v):
                    nc.sync.dma_start(out=tile_[0:2 * F], in_=src_v[0])
                    nc.sync.dma_start(out=tile_[64:64 + 2 * F], in_=src_v[1])

                xt0 = xts.tile([128, 8 * NT], bf16, tag="xt", name="xt0")
                xt_dma(xt0, xt_view[0])
                step(xt0, True, init=True)

                def double_step(src_a, src_b, last=False):
                    xt_a = xts.tile([128, 8 * NT], bf16, tag="xt",
                                    name="xt_a")
                    xt_dma(xt_a, src_a)
                    xt_b = xts.tile([128, 8 * NT], bf16, tag="xt",
                                    name="xt_b")
                    xt_dma(xt_b, src_b)
                    step(xt_a, False)
                    step(xt_b, False, last=last)

                KD = (T - 1) // 2
                if sim_steps is not None:
                    for k in range(sim_steps // 2):
                        double_step(xt_pairs[k][0], xt_pairs[k][1],
                                    last=(k == KD - 1))
                    heads_inline = sim_steps // 2 == KD
                elif unroll:
                    # fully unrolled scan: no For_i barriers, engines pipeline
                    # freely across steps; heads fold into the last step
                    for k in range(KD):
                        double_step(xt_pairs[k][0], xt_pairs[k][1],
                                    last=(k == KD - 1))
                    heads_inline = True
                else:
                    with tc.For_i(0, KD, 1,
                                  hint_engines=(nc.tensor.engine, nc.vector.engine,
                                                nc.scalar.engine)) as k:
                        double_step(xt_pairs[k][0], xt_pairs[k][1])
                    heads_inline = False

                if not heads_inline:
                    for p in range(NPAIR):
                        head(p)

            if repeat == 1:
                body()
            else:
                with tc.For_i(0, repeat, 1):
                    body()

    nc.finalize()
    return nc


def _get_nc(key):
    if key not in _BUILD_CACHE:
        _BUILD_CACHE[key] = _build_bass(*key)
    return _BUILD_CACHE[key]


def _prep_weights(inputs):
    # gate order permutation i,f,g,o -> i,f,o,g (sigmoid gates contiguous)
    def perm(n):
        return np.concatenate([np.arange(0, 2 * n), np.arange(3 * n, 4 * n),
                               np.arange(2 * n, 3 * n)])
    p0, p1 = perm(H1), perm(H2)

    w0ihT = inputs["l0_w_ih"][p0].T.astype(np.float32)     # [12, 256]
    w0hhT = inputs["l0_w_hh"][p0].T.astype(np.float32)     # [64, 256]
    w1ihT = inputs["l1_w_ih"][p1].T.astype(np.float32)     # [64, 512]
    w1hhT = inputs["l1_w_hh"][p1].T.astype(np.float32)     # [128, 512]

    # L0 ih block-diagonal, rows interleaved (f,half)
    w0ih_bd = np.zeros((2 * F, 512), np.float32)
    w0ih_bd[0::2, :] = np.concatenate(
        [np.pad(w0ihT[:, g * 64:(g + 1) * 64], [(0, 0), (0, 64)])
         for g in range(4)], axis=1)
    w0ih_bd[1::2, :] = np.concatenate(
        [np.pad(w0ihT[:, g * 64:(g + 1) * 64], [(0, 0), (64, 0)])
         for g in range(4)], axis=1)
    # L0 hh block-diagonal (A rows 0:64, B rows 64:128)
    w0hh_bd = np.zeros((2 * H1, 512), np.float32)
    for g in range(4):
        blk = w0hhT[:, g * 64:(g + 1) * 64]
        w0hh_bd[0:64, g * 128:g * 128 + 64] = blk
        w0hh_bd[64:128, g * 128 + 64:(g + 1) * 128] = blk
    # L1 ih half-masked (reads stacked h0)
    w1ih_A = np.concatenate([w1ihT, np.zeros_like(w1ihT)], axis=0)   # [128, 512]
    w1ih_B = np.concatenate([np.zeros_like(w1ihT), w1ihT], axis=0)
    fc1T = inputs["fc1_w"].T.astype(np.float32)            # [96, 64]
    fc1_A = np.concatenate([fc1T, np.zeros_like(fc1T)], axis=1)      # [96, 128]
    fc1_B = np.concatenate([np.zeros_like(fc1T), fc1T], axis=1)

    # replicate the L0 ih weights into the 4 partition groups (32-aligned)
    w0ih_rep = np.zeros((128, 512), np.float32)
    for g in range(2):
        w0ih_rep[64 * g:64 * g + 2 * F] = w0ih_bd

    wm = {
        "w0ih_bd": w0ih_rep.astype(BF16),
        "w0hh_bd": w0hh_bd.astype(BF16),
        "w1ih_A": w1ih_A.astype(BF16),
        "w1ih_B": w1ih_B.astype(BF16),
        "w1hhT": np.ascontiguousarray(w1hhT).astype(BF16),
        "fc1_A": fc1_A,
        "fc1_B": fc1_B,
        "fc2T": np.ascontiguousarray(inputs["fc2_w"].T).astype(np.float32),
        "d1T": np.ascontiguousarray(inputs["d1_w"].T).astype(BF16),
        "d2T": np.ascontiguousarray(inputs["d2_w"].T).astype(BF16),
    }

    b0 = (inputs["l0_b_ih"] + inputs["l0_b_hh"]).astype(np.float32)[p0]   # [256]
    b1 = (inputs["l1_b_ih"] + inputs["l1_b_hh"]).astype(np.float32)[p1]   # [512]
    gb = np.zeros((128, 8), np.float32)
    for g in range(4):
        gb[:, g] = np.tile(b0[g * 64:(g + 1) * 64], 2)     # stacked [A;B]
        gb[:, 4 + g] = b1[g * 128:(g + 1) * 128]
    vb = np.zeros((128, 4), np.float32)
    vb[:, 0] = np.tile(inputs["fc1_b"], 2)
    vb[:, 1] = inputs["fc2_b"]
    vb[0:H1, 2] = inputs["d1_b"]
    vb[0:1, 3] = inputs["d2_b"]
    wm["gate_bias"] = gb
    wm["vec_bias"] = vb
    has_gate_bias = bool(np.any(b0) or np.any(b1))
    has_vec_bias = bool(np.any(vb))
    return wm, has_gate_bias, has_vec_bias


def _in_maps(inputs, wm):
    x = np.asarray(inputs["input_seq"], dtype=np.float32)
    cs = np.asarray(inputs["cell_state"], dtype=np.float32)
    # xt[(p%2)*24 + 2f+h, t, (p//2)*512+n] = x[p*1024 + h*512 + n, t, f]
    xc = np.ascontiguousarray(x).reshape(NCORES, NPAIR // 2, 2, 2, NT, T, F)
    xr = xc.transpose(0, 2, 6, 3, 5, 1, 4)                  # [c,g,f,h,t,b,n]
    xt = np.ascontiguousarray(xr).reshape(
        NCORES, 2 * 2 * F, T, (NPAIR // 2) * NT).astype(BF16)
    # csT[p, k, h*512+n] = cs[p*1024 + h*512 + n, k]
    cc = cs.reshape(NCORES, NPAIR, 2, NT, CS)
    csT = np.ascontiguousarray(cc.transpose(0, 1, 4, 2, 3)).reshape(
        NCORES, NPAIR, CS, 2 * NT)
    maps = []
    for i in range(NCORES):
        m = dict(wm)
        m["xt"] = np.ascontiguousarray(xt[i])
        m["csT"] = np.ascontiguousarray(csT[i])
        maps.append(m)
    return maps


def kernel(**inputs):
    inputs = {k: np.asarray(v) for k, v in inputs.items()}
    wm, hgb, hvb = _prep_weights(inputs)
    nc = _get_nc((hgb, hvb))
    from concourse.bass_utils import run_bass_kernel_spmd
    res = run_bass_kernel_spmd(nc, _in_maps(inputs, wm),
                               core_ids=list(range(NCORES)))
    return np.concatenate([r["pred"] for r in res.results], axis=0)
